# revision 5
# baseline (speedup 1.0000x reference)
"""Trainium2 Bass kernel for causal multi-head attention (B=2, T=2048, D=2048, H=16).

Sharding: head-tensor-parallel across 8 cores — core c computes heads {2c, 2c+1}
for both batches (QKV projections, scores, softmax, PV). The channel-major
attention outputs are then redistributed with an 8-rank AllToAll (each core
sends, per target j, its two heads' columns for output-row slice j), after
which core c holds ALL 2048 attention channels for 512 flattened (b,t) output
rows and computes those rows of the output projection with N=512 matmuls.
The A2A moves ~1.9MB/core (vs 16MB for the AllGather formulation) and is
split into two collectives (one per local head) so it overlaps compute.

Other structure:
- All matmuls bf16 with fp32 PSUM accumulation. Scores computed transposed
  (S.T[tk,tq]) so softmax denominators are ones-matmuls and P.T feeds PV
  directly.
- Denominators: DVE pre-sums each group of 4 exp'd score tiles (bf16), then a
  single ones-matmul per group accumulates into one shared PSUM bank at
  partition offsets 32*tqb — 4x fewer denominator matmuls than per-tile.
- exp() emitted over up to 3 score tiles per activation instruction (score
  PSUM tile is [128,3,512]) to amortize ACT fixed overhead.
- No max-subtraction needed: scores ~N(0,1), far inside fp32/bf16 exp range.

`reps` emits the whole computation R times in one program (used by the test
harness to amplify device time above the axon dispatch floor).
"""

import numpy as np
import ml_dtypes

import concourse.bass as bass
import concourse.bacc as bacc
import concourse.mybir as mybir
import concourse.tile as tile
from concourse.bass_utils import run_bass_kernel_spmd

B, T, D, H, HD = 2, 2048, 2048, 16, 128
NCORES = 8
HPC = H // NCORES        # heads per core = 2
CW = HPC * HD            # channel slice per core = 256
NDT = D // 128           # 16 contraction tiles
NTQ = T // 512           # 4 query blocks of 512
NTK = T // 128           # 16 key tiles of 128
RPC = (B * T) // NCORES  # flattened output rows per core = 512
SCALE = 1.0 / float(np.sqrt(HD))

BF16 = mybir.dt.bfloat16
F32 = mybir.dt.float32
BF = ml_dtypes.bfloat16

_CACHE = {}

# lane order: head-local 0 of both batches first, then head-local 1 — so the
# first A2A (channels = every rank's head0) can fire halfway through phase 2.
LANES = [(0, 0), (1, 0), (0, 1), (1, 1)]  # (b, hl)


def _emit_rep(nc, tc, dram, params, rep):
    xt_p = params["xt"]          # [B][D, T] bf16 (query transposed)
    masks_sb = params["masks_sb"]
    wq_sb, wk_sb, wv_sb = params["wq_sb"], params["wk_sb"], params["wv_sb"]
    wo_sb = params["wo_sb"]
    ones_col = params["ones_col"]
    qh_sb, kh_sb, v_sb = params["qh_sb"], params["kh_sb"], params["v_sb"]
    out_p = params["out"]

    # DRAM staging for the two AllToAlls (one per local head).
    # cc_in_h[l] shard j (128 rows) = at((b(j), l), tqb=j%4)  [128ch, 512tq]
    cc_in = [dram.tile([NCORES * HD, 512], BF16, name=f"cc_in{l}_{rep}")
             for l in range(HPC)]
    cc_out = [dram.tile([NCORES * HD, 512], BF16, name=f"cc_out{l}_{rep}")
              for l in range(HPC)]

    # ---- Phase 1: QKV projections (per batch, per tq-half) ----
    with tc.tile_pool(name="p1", bufs=1) as p1, \
         tc.tile_pool(name="psum1", bufs=1, space="PSUM") as psum1:
        for b in range(B):
            xv = xt_p[b][:].rearrange("(n p) t -> p n t", p=128)
            for th in range(2):  # tq half
                xt_sb = p1.tile([128, NDT, T // 2], BF16, tag="xt", bufs=2,
                                name="xt_sb")
                for ch in range(4):
                    nc.sync.dma_start(
                        out=xt_sb[:, 4 * ch:4 * ch + 4, :],
                        in_=xv[:, 4 * ch:4 * ch + 4,
                               th * (T // 2):(th + 1) * (T // 2)])
                # Q.T / K.T per head: [hd=128, tq] — weight-stationary over
                # the tq-pair so LDWEIGHTS amortizes 2x.
                for hl in range(HPC):
                    lane = 2 * b + (0 if hl == 0 else 1)  # qh/kh lane index
                    for w_sb, dst in ((wq_sb, qh_sb), (wk_sb, kh_sb)):
                        ps = psum1.tile([128, 2, 512], F32, tag="qk", bufs=2,
                                        name="ps_qk")
                        for dt in range(NDT):
                            for tq in range(2):
                                nc.tensor.matmul(
                                    ps[:, tq, :],
                                    lhsT=w_sb[:, dt, hl * 128:(hl + 1) * 128],
                                    rhs=xt_sb[:, dt, tq * 512:(tq + 1) * 512],
                                    start=(dt == 0), stop=(dt == NDT - 1))
                        for tq in range(2):
                            tqg = th * 1024 + tq * 512
                            nc.vector.tensor_copy(
                                dst[:, lane, tqg:tqg + 512], ps[:, tq, :])
                # V in natural layout [tk, ch]
                for tkt in range(NTK // 2):
                    tkg = th * (NTK // 2) + tkt
                    ps = psum1.tile([128, CW], F32, tag="vproj", bufs=3,
                                    name="ps_v")
                    for dt in range(NDT):
                        nc.tensor.matmul(
                            ps[:],
                            lhsT=xt_sb[:, dt, tkt * 128:(tkt + 1) * 128],
                            rhs=wv_sb[:, dt, :],
                            start=(dt == 0), stop=(dt == NDT - 1))
                    nc.vector.tensor_copy(v_sb[:, b * NTK + tkg, :], ps[:])

    # ---- Phase 2: attention (lane order: head0 lanes first) + A2As ----
    with tc.tile_pool(name="p2", bufs=1) as p2, \
         tc.tile_pool(name="psum2", bufs=1, space="PSUM") as psum2:
        for li, (b, hl) in enumerate(LANES):
            lane = 2 * b + hl
            for tqb in range(NTQ):
                nkt = 4 * (tqb + 1)
                pt = p2.tile([128, NTK, 512], BF16, tag="pt", bufs=2,
                             name="pt")
                dn = psum2.tile([128, 512], F32, tag="dn", bufs=1, name="dn")
                ov = psum2.tile([128, 512], F32, tag="ov", bufs=1, name="ov")
                # scores in triples: [128,3,512] PSUM tile -> one exp over
                # up to 3 tiles
                ntrip = (nkt + 2) // 3
                for tr in range(ntrip):
                    k0 = 3 * tr
                    r = min(3, nkt - k0)
                    ps = psum2.tile([128, 3, 512], F32, tag="score", bufs=2,
                                    name="ps_s")
                    for i in range(r):
                        kt = k0 + i
                        nc.tensor.matmul(
                            ps[:, i, :],
                            lhsT=kh_sb[:, lane, kt * 128:(kt + 1) * 128],
                            rhs=qh_sb[:, lane, tqb * 512:(tqb + 1) * 512],
                            start=True, stop=True)
                    nc.scalar.activation(
                        pt[:, k0:k0 + r, :], ps[:, :r, :],
                        mybir.ActivationFunctionType.Exp, scale=SCALE)
                    # mask the diagonal-block tiles
                    for i in range(r):
                        kt = k0 + i
                        if kt >= 4 * tqb:
                            nc.vector.tensor_mul(
                                pt[:, kt, :], pt[:, kt, :],
                                masks_sb[:, kt - 4 * tqb, :])
                # PV accumulation over kt
                for kt in range(nkt):
                    nc.tensor.matmul(
                        ov[:],
                        lhsT=v_sb[:, b * NTK + kt, hl * 128:(hl + 1) * 128],
                        rhs=pt[:, kt, :],
                        start=(kt == 0), stop=(kt == nkt - 1))
                # denominator: quad-sum on DVE, then one ones-matmul per quad
                nquad = nkt // 4
                for q in range(nquad):
                    s4 = p2.tile([128, 512], BF16, tag="s4", bufs=2,
                                 name="s4")
                    t2 = p2.tile([128, 512], BF16, tag="t2", bufs=2,
                                 name="t2")
                    nc.vector.tensor_add(s4[:], pt[:, 4 * q, :],
                                         pt[:, 4 * q + 1, :])
                    nc.vector.tensor_add(t2[:], pt[:, 4 * q + 2, :],
                                         pt[:, 4 * q + 3, :])
                    nc.vector.tensor_add(s4[:], s4[:], t2[:])
                    nc.tensor.matmul(
                        dn[0:1, :],
                        lhsT=ones_col[:], rhs=s4[:],
                        start=(q == 0), stop=(q == nquad - 1))
                rc = p2.tile([1, 512], F32, tag="rc", bufs=2, name="rc")
                nc.vector.reciprocal(rc[:], dn[0:1, :])
                bc = p2.tile([128, 512], F32, tag="bc", bufs=2, name="bc")
                nc.gpsimd.partition_broadcast(bc[:], rc[:])
                at = p2.tile([128, 512], BF16, tag="at", bufs=3, name="at")
                nc.vector.tensor_mul(at[:], ov[:], bc[:])
                # shard j of cc_in[hl]: j = 4*b + tqb
                j = 4 * b + tqb
                nc.sync.dma_start(
                    out=cc_in[hl][j * 128:(j + 1) * 128, :], in_=at[:])
            if li == 1:  # both head0 lanes done -> first A2A
                nc.gpsimd.collective_compute(
                    "AllToAll", mybir.AluOpType.bypass,
                    replica_groups=[list(range(NCORES))],
                    ins=[cc_in[0][:]], outs=[cc_out[0][:]])
        nc.gpsimd.collective_compute(
            "AllToAll", mybir.AluOpType.bypass,
            replica_groups=[list(range(NCORES))],
            ins=[cc_in[1][:]], outs=[cc_out[1][:]])

    # ---- Phase 3: output projection for my 512 flattened rows ----
    # A_all channel blocks: ct 0..7  = rank i's head0 (global heads 0,2,..,14)
    #                       ct 8..15 = rank i's head1 (global heads 1,3,..,15)
    # wo_sb rows are host-permuted to match.
    with tc.tile_pool(name="p3", bufs=1) as p3, \
         tc.tile_pool(name="psum3", bufs=1, space="PSUM") as psum3:
        a_sb = [None, None]
        for l in range(HPC):
            a_sb[l] = p3.tile([128, NCORES, 512], BF16, name=f"a_sb{l}")
            cv = cc_out[l][:].rearrange("(ct p) t -> p ct t", p=128)
            for i in range(NCORES):
                nc.sync.dma_start(out=a_sb[l][:, i, :], in_=cv[:, i, :])
        for tp in range(2):  # tq-tile pairs (2 x 128 rows of my 512)
            po = [psum3.tile([128, 4, 512], F32, tag=f"po{i}", bufs=1,
                             name=f"po{i}") for i in range(2)]
            for l in range(HPC):  # ct halves, gated on A2A l
                for ct in range(NCORES):
                    for i in range(2):  # tqt within pair
                        tqt = 2 * tp + i
                        for cwg in range(4):
                            nc.tensor.matmul(
                                po[i][:, cwg, :],
                                lhsT=a_sb[l][:, ct,
                                             tqt * 128:(tqt + 1) * 128],
                                rhs=wo_sb[:, l * NCORES + ct,
                                          cwg * 512:(cwg + 1) * 512],
                                start=(l == 0 and ct == 0),
                                stop=(l == 1 and ct == NCORES - 1))
            for i in range(2):
                tqt = 2 * tp + i
                ot = p3.tile([128, 4, 512], F32, tag="ot", bufs=2, name="ot")
                nc.vector.tensor_copy(ot[:], po[i][:])
                nc.sync.dma_start(
                    out=out_p[tqt * 128:(tqt + 1) * 128, :],
                    in_=ot[:].rearrange("p i j -> p (i j)"))


def _build(reps: int = 1):
    nc = bacc.Bacc("TRN2", target_bir_lowering=False, debug=False,
                   num_devices=NCORES)

    params = {}
    params["xt"] = [nc.declare_dram_parameter(f"xt{b}", [D, T], BF16,
                                              isOutput=False)
                    for b in range(B)]
    for w in ("wq", "wk", "wv"):
        params[w] = nc.declare_dram_parameter(w, [D, CW], BF16,
                                              isOutput=False)
    params["wo"] = nc.declare_dram_parameter("wo", [D, D], BF16,
                                             isOutput=False)
    params["masks"] = nc.declare_dram_parameter("masks", [4, 128, 512], BF16,
                                                isOutput=False)
    params["out"] = nc.declare_dram_parameter("out", [RPC, D], F32,
                                              isOutput=True)

    with tile.TileContext(nc) as tc:
        with tc.tile_pool(name="consts", bufs=1) as consts, \
             tc.tile_pool(name="qkv", bufs=1) as qkv, \
             tc.tile_pool(name="dram", bufs=1, space="DRAM") as dram:

            masks_sb = consts.tile([128, 4, 512], BF16, name="masks_sb")
            nc.sync.dma_start(out=masks_sb[:],
                              in_=params["masks"][:].rearrange(
                                  "i p j -> p i j"))
            params["masks_sb"] = masks_sb
            for w, nm in (("wq", "wq_sb"), ("wk", "wk_sb"), ("wv", "wv_sb")):
                w_sb = consts.tile([128, NDT, CW], BF16, name=nm)
                wv_ = params[w][:].rearrange("(n p) j -> p n j", p=128)
                for ch in range(4):
                    nc.sync.dma_start(out=w_sb[:, 4 * ch:4 * ch + 4, :],
                                      in_=wv_[:, 4 * ch:4 * ch + 4, :])
                params[nm] = w_sb
            wo_sb = consts.tile([128, NDT, D], BF16, name="wo_sb")
            wov = params["wo"][:].rearrange("(n p) j -> p n j", p=128)
            for ch in range(8):
                nc.sync.dma_start(out=wo_sb[:, 2 * ch:2 * ch + 2, :],
                                  in_=wov[:, 2 * ch:2 * ch + 2, :])
            params["wo_sb"] = wo_sb
            ones_col = consts.tile([128, 1], BF16, name="ones_col")
            nc.vector.memset(ones_col[:], 1.0)
            params["ones_col"] = ones_col

            # channel-major Q.T/K.T + natural V, resident through phase 2
            params["qh_sb"] = qkv.tile([128, B * HPC, T], BF16, name="qh_sb")
            params["kh_sb"] = qkv.tile([128, B * HPC, T], BF16, name="kh_sb")
            params["v_sb"] = qkv.tile([128, B * NTK, CW], BF16, name="v_sb")

            for rep in range(reps):
                _emit_rep(nc, tc, dram, params, rep)

    nc.compile()
    return nc


def _get_nc(reps: int = 1):
    key = f"nc{reps}"
    if key not in _CACHE:
        _CACHE[key] = _build(reps)
    return _CACHE[key]


def make_in_maps(query, Wq, Wk, Wv, Wo):
    """Per-core input maps (shared host-side prep for kernel() and test)."""
    query = np.asarray(query, dtype=np.float32)
    xt = [np.ascontiguousarray(query[b].T).astype(BF) for b in range(B)]
    p_idx = np.arange(128)[:, None]
    j_idx = np.arange(512)[None, :]
    masks = np.stack([(p_idx <= j_idx - 128 * i) for i in range(4)]
                     ).astype(BF)
    # Wo rows (input channels) permuted to the A2A channel-block order:
    # heads [0,2,...,14, 1,3,...,15]
    woT = np.ascontiguousarray(np.asarray(Wo, np.float32).T)
    perm = [h for h in range(0, H, 2)] + [h for h in range(1, H, 2)]
    wo_perm = np.concatenate([woT[128 * h:128 * (h + 1)] for h in perm],
                             axis=0).astype(BF)
    in_maps = []
    for c in range(NCORES):
        sl = slice(CW * c, CW * (c + 1))
        in_maps.append({
            "xt0": xt[0],
            "xt1": xt[1],
            "wq": np.ascontiguousarray(np.asarray(Wq, np.float32)[sl].T
                                       ).astype(BF),
            "wk": np.ascontiguousarray(np.asarray(Wk, np.float32)[sl].T
                                       ).astype(BF),
            "wv": np.ascontiguousarray(np.asarray(Wv, np.float32)[sl].T
                                       ).astype(BF),
            "wo": wo_perm,
            "masks": masks,
        })
    return in_maps


def kernel(query, attention_mask, Wq, Wk, Wv, Wo, bo):
    nc = _get_nc()
    in_maps = make_in_maps(query, Wq, Wk, Wv, Wo)
    res = run_bass_kernel_spmd(nc, in_maps, list(range(NCORES))).results
    flat = np.concatenate([res[c]["out"] for c in range(NCORES)], axis=0)
    out = flat.reshape(B, T, D) + np.asarray(bo, np.float32)[None, None, :]
    return out.astype(np.float32)


# revision 6
# speedup vs baseline: 1.1146x; 1.1146x over previous
"""Trainium2 Bass kernel for causal multi-head attention (B=2, T=2048, D=2048, H=16).

Sharding: head-tensor-parallel across 8 cores — core c computes heads {2c, 2c+1}
for both batches (QKV projections, scores, softmax, PV). The channel-major
attention outputs are then redistributed with an 8-rank AllToAll (each core
sends, per target j, its two heads' columns for output-row slice j), after
which core c holds ALL 2048 attention channels for 512 flattened (b,t) output
rows and computes those rows of the output projection with N=512 matmuls.
The A2A moves ~2MB/core (vs 16MB for an AllGather formulation) and is split
into two collectives (one per local head) so it overlaps compute.

Performance structure (from perfetto traces):
- Phase 1 runs at ~100% PE busy; keep its shape. Wo (8MB) is NOT loaded at
  program start — its DMA is emitted at the start of phase 2 so the query
  loads aren't queued behind it (saves ~30us of PE idle at startup).
- Phase 2 processes query blocks in DESCENDING size order with double-buffered
  ov/dn PSUM so each block's softmax serial chain (exp -> mask -> pair-sum ->
  ones-matmul -> reciprocal -> broadcast -> normalize) hides under the next
  block's score matmuls. This also keeps the PE dense so HAM stays at 2.4GHz.
- Scores are computed in pairs ([128,2,512] PSUM) with one exp() per pair and
  PV matmuls interleaved right after each pair's exp.
- Softmax denominators: DVE sums each exp'd pair, then one ones-matmul per
  pair accumulates the column sums (half the denominator matmul streaming).
- Phase 3 runs in two passes: the ct-blocks from A2A#1 are contracted into
  PSUM and drained to SBUF as f32 partials while A2A#2 is still in flight;
  the second pass contracts A2A#2's blocks and a DVE add merges the partials.
"""

import numpy as np
import ml_dtypes

import concourse.bass as bass
import concourse.bacc as bacc
import concourse.mybir as mybir
import concourse.tile as tile
from concourse.bass_utils import run_bass_kernel_spmd

B, T, D, H, HD = 2, 2048, 2048, 16, 128
NCORES = 8
HPC = H // NCORES        # heads per core = 2
CW = HPC * HD            # channel slice per core = 256
NDT = D // 128           # 16 contraction tiles
NTQ = T // 512           # 4 query blocks of 512
NTK = T // 128           # 16 key tiles of 128
RPC = (B * T) // NCORES  # flattened output rows per core = 512
SCALE = 1.0 / float(np.sqrt(HD))

BF16 = mybir.dt.bfloat16
F32 = mybir.dt.float32
BF = ml_dtypes.bfloat16

_CACHE = {}

# lane order: head-local 0 of both batches first, then head-local 1 — so the
# first A2A (channels = every rank's head0) can fire halfway through phase 2.
LANES = [(0, 0), (1, 0), (0, 1), (1, 1)]  # (b, hl)


def _load_weights(nc, params):
    """Emit the wq/wk/wv SBUF loads (phase-1 weights, needed early)."""
    for w, nm in (("wq", "wq_sb"), ("wk", "wk_sb"), ("wv", "wv_sb")):
        wv_ = params[w][:].rearrange("(n p) j -> p n j", p=128)
        w_sb = params[nm]
        for ch in range(4):
            nc.sync.dma_start(out=w_sb[:, 4 * ch:4 * ch + 4, :],
                              in_=wv_[:, 4 * ch:4 * ch + 4, :])


def _emit_rep(nc, tc, dram, params, rep):
    xt_p = params["xt"]          # [B][D, T] bf16 (query transposed)
    masks_sb = params["masks_sb"]
    wq_sb, wk_sb, wv_sb = params["wq_sb"], params["wk_sb"], params["wv_sb"]
    wo_sb = params["wo_sb"]
    ones_col = params["ones_col"]
    qh_sb, kh_sb, v_sb = params["qh_sb"], params["kh_sb"], params["v_sb"]
    out_p = params["out"]

    # DRAM staging for the two AllToAlls (one per local head).
    # cc_in[l] shard j (128 rows) = at((b(j), l), tqb=j%4)  [128ch, 512tq]
    cc_in = [dram.tile([NCORES * HD, 512], BF16, name=f"cc_in{l}_{rep}")
             for l in range(HPC)]
    cc_out = [dram.tile([NCORES * HD, 512], BF16, name=f"cc_out{l}_{rep}")
              for l in range(HPC)]

    # ---- Phase 1: QKV projections (per batch, per tq-half) ----
    with tc.tile_pool(name="p1", bufs=1) as p1, \
         tc.tile_pool(name="psum1", bufs=1, space="PSUM") as psum1:
        for b in range(B):
            xv = xt_p[b][:].rearrange("(n p) t -> p n t", p=128)
            for th in range(2):  # tq half
                xt_sb = p1.tile([128, NDT, T // 2], BF16, tag="xt", bufs=2,
                                name="xt_sb")
                for ch in range(4):
                    nc.sync.dma_start(
                        out=xt_sb[:, 4 * ch:4 * ch + 4, :],
                        in_=xv[:, 4 * ch:4 * ch + 4,
                               th * (T // 2):(th + 1) * (T // 2)])
                if b == 0 and th == 0 and rep == 0:
                    _load_weights(nc, params)
                # Q.T / K.T per head: [hd=128, tq]
                for hl in range(HPC):
                    lane = 2 * b + hl
                    for w_sb, dst in ((wq_sb, qh_sb), (wk_sb, kh_sb)):
                        ps = psum1.tile([128, 2, 512], F32, tag="qk", bufs=2,
                                        name="ps_qk")
                        for dt in range(NDT):
                            for tq in range(2):
                                nc.tensor.matmul(
                                    ps[:, tq, :],
                                    lhsT=w_sb[:, dt, hl * 128:(hl + 1) * 128],
                                    rhs=xt_sb[:, dt, tq * 512:(tq + 1) * 512],
                                    start=(dt == 0), stop=(dt == NDT - 1))
                        tqg = th * 1024
                        nc.vector.tensor_copy(
                            dst[:, lane, tqg:tqg + 1024],
                            ps[:].rearrange("p i j -> p (i j)"))
                # V in natural layout [tk, ch]
                for tkt in range(NTK // 2):
                    tkg = th * (NTK // 2) + tkt
                    ps = psum1.tile([128, CW], F32, tag="vproj", bufs=3,
                                    name="ps_v")
                    for dt in range(NDT):
                        nc.tensor.matmul(
                            ps[:],
                            lhsT=xt_sb[:, dt, tkt * 128:(tkt + 1) * 128],
                            rhs=wv_sb[:, dt, :],
                            start=(dt == 0), stop=(dt == NDT - 1))
                    nc.vector.tensor_copy(v_sb[:, b * NTK + tkg, :], ps[:])

    # ---- Phase 2: attention (lane order: head0 lanes first) + A2As ----
    with tc.tile_pool(name="p2", bufs=1) as p2, \
         tc.tile_pool(name="psum2", bufs=1, space="PSUM") as psum2:
        if rep == 0:
            # Wo load now — after phase-1 query DMAs, well before phase 3.
            wov = params["wo"][:].rearrange("(n p) j -> p n j", p=128)
            for ch in range(8):
                nc.sync.dma_start(out=wo_sb[:, 2 * ch:2 * ch + 2, :],
                                  in_=wov[:, 2 * ch:2 * ch + 2, :])
        for li, (b, hl) in enumerate(LANES):
            lane = 2 * b + hl
            for tqb in reversed(range(NTQ)):  # descending block size
                nkt = 4 * (tqb + 1)
                npair = nkt // 2
                pt = p2.tile([128, NTK, 512], BF16, tag="pt", bufs=2,
                             name="pt")
                dn = psum2.tile([128, 512], F32, tag="dn", bufs=2, name="dn")
                ov = psum2.tile([128, 512], F32, tag="ov", bufs=2, name="ov")
                for pr in range(npair):
                    k0 = 2 * pr
                    ps = psum2.tile([128, 2, 512], F32, tag="score", bufs=2,
                                    name="ps_s")
                    for i in range(2):
                        kt = k0 + i
                        nc.tensor.matmul(
                            ps[:, i, :],
                            lhsT=kh_sb[:, lane, kt * 128:(kt + 1) * 128],
                            rhs=qh_sb[:, lane, tqb * 512:(tqb + 1) * 512],
                            start=True, stop=True)
                    nc.scalar.activation(
                        pt[:, k0:k0 + 2, :], ps[:],
                        mybir.ActivationFunctionType.Exp, scale=SCALE)
                    if k0 >= 4 * tqb:  # diagonal-block pair: apply mask
                        mi = k0 - 4 * tqb
                        nc.vector.tensor_mul(
                            pt[:, k0:k0 + 2, :].rearrange("p i j -> p (i j)"),
                            pt[:, k0:k0 + 2, :].rearrange("p i j -> p (i j)"),
                            masks_sb[:, mi:mi + 2, :].rearrange(
                                "p i j -> p (i j)"))
                    # PV for this pair
                    for i in range(2):
                        kt = k0 + i
                        nc.tensor.matmul(
                            ov[:],
                            lhsT=v_sb[:, b * NTK + kt,
                                      hl * 128:(hl + 1) * 128],
                            rhs=pt[:, kt, :],
                            start=(kt == 0), stop=(kt == nkt - 1))
                    # denominator contribution of this pair
                    s2 = p2.tile([128, 512], BF16, tag="s2", bufs=3,
                                 name="s2")
                    nc.vector.tensor_add(s2[:], pt[:, k0, :],
                                         pt[:, k0 + 1, :])
                    nc.tensor.matmul(
                        dn[0:1, :], lhsT=ones_col[:], rhs=s2[:],
                        start=(pr == 0), stop=(pr == npair - 1))
                rc = p2.tile([1, 512], F32, tag="rc", bufs=2, name="rc")
                nc.vector.reciprocal(rc[:], dn[0:1, :])
                bc = p2.tile([128, 512], F32, tag="bc", bufs=2, name="bc")
                nc.gpsimd.partition_broadcast(bc[:], rc[:])
                at = p2.tile([128, 512], BF16, tag="at", bufs=3, name="at")
                nc.vector.tensor_mul(at[:], ov[:], bc[:])
                j = 4 * b + tqb
                nc.sync.dma_start(
                    out=cc_in[hl][j * 128:(j + 1) * 128, :], in_=at[:])
            if li == 1:  # both head0 lanes done -> first A2A
                nc.gpsimd.collective_compute(
                    "AllToAll", mybir.AluOpType.bypass,
                    replica_groups=[list(range(NCORES))],
                    ins=[cc_in[0][:]], outs=[cc_out[0][:]])
        nc.gpsimd.collective_compute(
            "AllToAll", mybir.AluOpType.bypass,
            replica_groups=[list(range(NCORES))],
            ins=[cc_in[1][:]], outs=[cc_out[1][:]])

    # ---- Phase 3: output projection for my 512 flattened rows ----
    # A_all channel blocks: pass l=0 -> global heads 0,2,..,14 (A2A#1),
    # pass l=1 -> heads 1,3,..,15 (A2A#2). wo_sb rows host-permuted to match.
    with tc.tile_pool(name="p3", bufs=1) as p3, \
         tc.tile_pool(name="psum3", bufs=1, space="PSUM") as psum3:
        a_sb = [None, None]
        part = [[None] * 2 for _ in range(4)]  # [tqt][cwh] f32 partials
        for l in range(HPC):
            a_sb[l] = p3.tile([128, NCORES, 512], BF16, name=f"a_sb{l}")
            cv = cc_out[l][:].rearrange("(ct p) t -> p ct t", p=128)
            for i in range(NCORES):
                nc.sync.dma_start(out=a_sb[l][:, i, :], in_=cv[:, i, :])
            for tqt in range(4):
                for cwh in range(2):
                    po = psum3.tile([128, 2, 512], F32, tag="po", bufs=3,
                                    name="po")
                    for ct in range(NCORES):
                        for cwg in range(2):
                            cw0 = cwh * 1024 + cwg * 512
                            nc.tensor.matmul(
                                po[:, cwg, :],
                                lhsT=a_sb[l][:, ct,
                                             tqt * 128:(tqt + 1) * 128],
                                rhs=wo_sb[:, l * NCORES + ct,
                                          cw0:cw0 + 512],
                                start=(ct == 0), stop=(ct == NCORES - 1))
                    if l == 0:
                        pp = p3.tile([128, 2, 512], F32,
                                     tag=f"part{tqt}_{cwh}", bufs=1,
                                     name=f"part{tqt}_{cwh}")
                        part[tqt][cwh] = pp
                        nc.vector.tensor_copy(pp[:], po[:])
                    else:
                        ot = p3.tile([128, 2, 512], F32, tag="ot", bufs=3,
                                     name="ot")
                        nc.vector.tensor_add(ot[:], po[:],
                                             part[tqt][cwh][:])
                        nc.sync.dma_start(
                            out=out_p[tqt * 128:(tqt + 1) * 128,
                                      cwh * 1024:(cwh + 1) * 1024],
                            in_=ot[:].rearrange("p i j -> p (i j)"))


def _build(reps: int = 1):
    nc = bacc.Bacc("TRN2", target_bir_lowering=False, debug=False,
                   num_devices=NCORES)

    params = {}
    params["xt"] = [nc.declare_dram_parameter(f"xt{b}", [D, T], BF16,
                                              isOutput=False)
                    for b in range(B)]
    for w in ("wq", "wk", "wv"):
        params[w] = nc.declare_dram_parameter(w, [D, CW], BF16,
                                              isOutput=False)
    params["wo"] = nc.declare_dram_parameter("wo", [D, D], BF16,
                                             isOutput=False)
    params["masks"] = nc.declare_dram_parameter("masks", [4, 128, 512], BF16,
                                                isOutput=False)
    params["out"] = nc.declare_dram_parameter("out", [RPC, D], F32,
                                              isOutput=True)

    with tile.TileContext(nc) as tc:
        with tc.tile_pool(name="consts", bufs=1) as consts, \
             tc.tile_pool(name="qkv", bufs=1) as qkv, \
             tc.tile_pool(name="dram", bufs=1, space="DRAM") as dram:

            for w, nm in (("wq", "wq_sb"), ("wk", "wk_sb"), ("wv", "wv_sb")):
                params[nm] = consts.tile([128, NDT, CW], BF16, name=nm)
            params["wo_sb"] = consts.tile([128, NDT, D], BF16, name="wo_sb")
            masks_sb = consts.tile([128, 4, 512], BF16, name="masks_sb")
            nc.sync.dma_start(out=masks_sb[:],
                              in_=params["masks"][:].rearrange(
                                  "i p j -> p i j"))
            params["masks_sb"] = masks_sb
            ones_col = consts.tile([128, 1], BF16, name="ones_col")
            nc.vector.memset(ones_col[:], 1.0)
            params["ones_col"] = ones_col

            # channel-major Q.T/K.T + natural V, resident through phase 2
            params["qh_sb"] = qkv.tile([128, B * HPC, T], BF16, name="qh_sb")
            params["kh_sb"] = qkv.tile([128, B * HPC, T], BF16, name="kh_sb")
            params["v_sb"] = qkv.tile([128, B * NTK, CW], BF16, name="v_sb")

            for rep in range(reps):
                _emit_rep(nc, tc, dram, params, rep)

    nc.compile()
    return nc


def _get_nc(reps: int = 1):
    key = f"nc{reps}"
    if key not in _CACHE:
        _CACHE[key] = _build(reps)
    return _CACHE[key]


def make_in_maps(query, Wq, Wk, Wv, Wo):
    """Per-core input maps (shared host-side prep for kernel() and test)."""
    query = np.asarray(query, dtype=np.float32)
    xt = [np.ascontiguousarray(query[b].T).astype(BF) for b in range(B)]
    p_idx = np.arange(128)[:, None]
    j_idx = np.arange(512)[None, :]
    masks = np.stack([(p_idx <= j_idx - 128 * i) for i in range(4)]
                     ).astype(BF)
    # Wo rows (input channels) permuted to the A2A channel-block order:
    # heads [0,2,...,14, 1,3,...,15]
    woT = np.ascontiguousarray(np.asarray(Wo, np.float32).T)
    perm = [h for h in range(0, H, 2)] + [h for h in range(1, H, 2)]
    wo_perm = np.concatenate([woT[128 * h:128 * (h + 1)] for h in perm],
                             axis=0).astype(BF)
    in_maps = []
    for c in range(NCORES):
        sl = slice(CW * c, CW * (c + 1))
        in_maps.append({
            "xt0": xt[0],
            "xt1": xt[1],
            "wq": np.ascontiguousarray(np.asarray(Wq, np.float32)[sl].T
                                       ).astype(BF),
            "wk": np.ascontiguousarray(np.asarray(Wk, np.float32)[sl].T
                                       ).astype(BF),
            "wv": np.ascontiguousarray(np.asarray(Wv, np.float32)[sl].T
                                       ).astype(BF),
            "wo": wo_perm,
            "masks": masks,
        })
    return in_maps


def kernel(query, attention_mask, Wq, Wk, Wv, Wo, bo):
    nc = _get_nc()
    in_maps = make_in_maps(query, Wq, Wk, Wv, Wo)
    res = run_bass_kernel_spmd(nc, in_maps, list(range(NCORES))).results
    flat = np.concatenate([res[c]["out"] for c in range(NCORES)], axis=0)
    out = flat.reshape(B, T, D) + np.asarray(bo, np.float32)[None, None, :]
    return out.astype(np.float32)


# revision 7
# speedup vs baseline: 1.1480x; 1.0300x over previous
"""Trainium2 Bass kernel for causal multi-head attention (B=2, T=2048, D=2048, H=16).

Sharding: head-tensor-parallel across 8 cores — core c computes heads {2c, 2c+1}
for both batches (QKV projections, scores, softmax, PV). The channel-major
attention outputs are then redistributed with an 8-rank AllToAll (each core
sends, per target j, its two heads' columns for output-row slice j), after
which core c holds ALL 2048 attention channels for 512 flattened (b,t) output
rows and computes those rows of the output projection with N=512 matmuls.
The A2A moves ~2MB/core (vs 16MB for an AllGather formulation) and is split
into two collectives (one per local head) so it overlaps compute.

Performance structure (from perfetto traces):
- Phase 1 runs at ~100% PE busy; keep its shape. Wo (8MB) is NOT loaded at
  program start — its DMA is emitted at the start of phase 2 so the query
  loads aren't queued behind it (saves ~30us of PE idle at startup).
- Phase 2 processes query blocks in DESCENDING size order with double-buffered
  ov/dn PSUM so each block's softmax serial chain (exp -> mask -> pair-sum ->
  ones-matmul -> reciprocal -> broadcast -> normalize) hides under the next
  block's score matmuls. This also keeps the PE dense so HAM stays at 2.4GHz.
- Scores are computed in pairs ([128,2,512] PSUM) with one exp() per pair and
  PV matmuls interleaved right after each pair's exp.
- Softmax denominators: DVE sums each exp'd pair, then one ones-matmul per
  pair accumulates the column sums (half the denominator matmul streaming).
- Phase 3 runs in two passes: the ct-blocks from A2A#1 are contracted into
  PSUM and drained to SBUF as f32 partials while A2A#2 is still in flight;
  the second pass contracts A2A#2's blocks and a DVE add merges the partials.
"""

import numpy as np
import ml_dtypes

import concourse.bass as bass
import concourse.bacc as bacc
import concourse.mybir as mybir
import concourse.tile as tile
from concourse.bass_utils import run_bass_kernel_spmd

B, T, D, H, HD = 2, 2048, 2048, 16, 128
NCORES = 8
HPC = H // NCORES        # heads per core = 2
CW = HPC * HD            # channel slice per core = 256
NDT = D // 128           # 16 contraction tiles
NTQ = T // 512           # 4 query blocks of 512
NTK = T // 128           # 16 key tiles of 128
RPC = (B * T) // NCORES  # flattened output rows per core = 512
SCALE = 1.0 / float(np.sqrt(HD))

BF16 = mybir.dt.bfloat16
F32 = mybir.dt.float32
BF = ml_dtypes.bfloat16

_CACHE = {}

# lane order: head-local 0 of both batches first, then head-local 1 — so the
# first A2A (channels = every rank's head0) can fire halfway through phase 2.
LANES = [(0, 0), (1, 0), (0, 1), (1, 1)]  # (b, hl)


def _load_weights(nc, params):
    """Emit the wq/wk/wv SBUF loads (phase-1 weights, needed early)."""
    for w, nm in (("wq", "wq_sb"), ("wk", "wk_sb"), ("wv", "wv_sb")):
        wv_ = params[w][:].rearrange("(n p) j -> p n j", p=128)
        w_sb = params[nm]
        for ch in range(4):
            nc.sync.dma_start(out=w_sb[:, 4 * ch:4 * ch + 4, :],
                              in_=wv_[:, 4 * ch:4 * ch + 4, :])


def _emit_rep(nc, tc, dram, params, rep):
    xt_p = params["xt"]          # [B][D, T] bf16 (query transposed)
    masks_sb = params["masks_sb"]
    wq_sb, wk_sb, wv_sb = params["wq_sb"], params["wk_sb"], params["wv_sb"]
    wo_sb = params["wo_sb"]
    ones_col = params["ones_col"]
    qh_sb, kh_sb, v_sb = params["qh_sb"], params["kh_sb"], params["v_sb"]
    out_p = params["out"]

    # DRAM staging for the two AllToAlls (one per local head).
    # cc_in[l] shard j (128 rows) = at((b(j), l), tqb=j%4)  [128ch, 512tq]
    cc_in = [dram.tile([NCORES * HD, 512], BF16, name=f"cc_in{l}_{rep}")
             for l in range(HPC)]
    cc_out = [dram.tile([NCORES * HD, 512], BF16, name=f"cc_out{l}_{rep}")
              for l in range(HPC)]

    # ---- Phase 1: QKV projections (per batch, per tq-half) ----
    with tc.tile_pool(name="p1", bufs=1) as p1, \
         tc.tile_pool(name="psum1", bufs=1, space="PSUM") as psum1:
        for b in range(B):
            xv = xt_p[b][:].rearrange("(n p) t -> p n t", p=128)
            for th in range(2):  # tq half
                xt_sb = p1.tile([128, NDT, T // 2], BF16, tag="xt", bufs=2,
                                name="xt_sb")
                for ch in range(4):
                    nc.sync.dma_start(
                        out=xt_sb[:, 4 * ch:4 * ch + 4, :],
                        in_=xv[:, 4 * ch:4 * ch + 4,
                               th * (T // 2):(th + 1) * (T // 2)])
                if b == 0 and th == 0 and rep == 0:
                    _load_weights(nc, params)
                # Q.T / K.T per head: [hd=128, tq]
                for hl in range(HPC):
                    lane = 2 * b + hl
                    for w_sb, dst in ((wq_sb, qh_sb), (wk_sb, kh_sb)):
                        ps = psum1.tile([128, 2, 512], F32, tag="qk", bufs=2,
                                        name="ps_qk")
                        for dt in range(NDT):
                            for tq in range(2):
                                nc.tensor.matmul(
                                    ps[:, tq, :],
                                    lhsT=w_sb[:, dt, hl * 128:(hl + 1) * 128],
                                    rhs=xt_sb[:, dt, tq * 512:(tq + 1) * 512],
                                    start=(dt == 0), stop=(dt == NDT - 1))
                        tqg = th * 1024
                        nc.vector.tensor_copy(
                            dst[:, lane, tqg:tqg + 1024],
                            ps[:].rearrange("p i j -> p (i j)"))
                # V in natural layout [tk, ch]
                for tkt in range(NTK // 2):
                    tkg = th * (NTK // 2) + tkt
                    ps = psum1.tile([128, CW], F32, tag="vproj", bufs=3,
                                    name="ps_v")
                    for dt in range(NDT):
                        nc.tensor.matmul(
                            ps[:],
                            lhsT=xt_sb[:, dt, tkt * 128:(tkt + 1) * 128],
                            rhs=wv_sb[:, dt, :],
                            start=(dt == 0), stop=(dt == NDT - 1))
                    nc.vector.tensor_copy(v_sb[:, b * NTK + tkg, :], ps[:])

    # ---- Phase 2: attention (lane order: head0 lanes first) + A2As ----
    with tc.tile_pool(name="p2", bufs=1) as p2, \
         tc.tile_pool(name="psum2", bufs=1, space="PSUM") as psum2:
        if rep == 0:
            # Wo load now — after phase-1 query DMAs, well before phase 3.
            wov = params["wo"][:].rearrange("(n p) j -> p n j", p=128)
            for ch in range(8):
                nc.sync.dma_start(out=wo_sb[:, 2 * ch:2 * ch + 2, :],
                                  in_=wov[:, 2 * ch:2 * ch + 2, :])
        for li, (b, hl) in enumerate(LANES):
            lane = 2 * b + hl
            for tqb in reversed(range(NTQ)):  # descending block size
                nkt = 4 * (tqb + 1)
                npair = nkt // 2
                pt = p2.tile([128, NTK, 512], BF16, tag="pt", bufs=2,
                             name="pt")
                dn = psum2.tile([128, 512], F32, tag="dn", bufs=2, name="dn")
                ov = psum2.tile([128, 512], F32, tag="ov", bufs=2, name="ov")

                def _pv_dn(pr):
                    # PV + denominator for pair pr (issued one pair late so
                    # the PE never waits on exp of the pair it just scored)
                    k0 = 2 * pr
                    for i in range(2):
                        kt = k0 + i
                        nc.tensor.matmul(
                            ov[:],
                            lhsT=v_sb[:, b * NTK + kt,
                                      hl * 128:(hl + 1) * 128],
                            rhs=pt[:, kt, :],
                            start=(kt == 0), stop=(kt == nkt - 1))
                    s2 = p2.tile([128, 512], BF16, tag="s2", bufs=3,
                                 name="s2")
                    nc.vector.tensor_add(s2[:], pt[:, k0, :],
                                         pt[:, k0 + 1, :])
                    nc.tensor.matmul(
                        dn[0:1, :], lhsT=ones_col[:], rhs=s2[:],
                        start=(pr == 0), stop=(pr == npair - 1))

                for pr in range(npair):
                    k0 = 2 * pr
                    ps = psum2.tile([128, 2, 512], F32, tag="score", bufs=2,
                                    name="ps_s")
                    for i in range(2):
                        kt = k0 + i
                        nc.tensor.matmul(
                            ps[:, i, :],
                            lhsT=kh_sb[:, lane, kt * 128:(kt + 1) * 128],
                            rhs=qh_sb[:, lane, tqb * 512:(tqb + 1) * 512],
                            start=True, stop=True)
                    if pr > 0:
                        _pv_dn(pr - 1)
                    nc.scalar.activation(
                        pt[:, k0:k0 + 2, :], ps[:],
                        mybir.ActivationFunctionType.Exp, scale=SCALE)
                    if k0 >= 4 * tqb:  # diagonal-block pair: apply mask
                        mi = k0 - 4 * tqb
                        nc.vector.tensor_mul(
                            pt[:, k0:k0 + 2, :].rearrange("p i j -> p (i j)"),
                            pt[:, k0:k0 + 2, :].rearrange("p i j -> p (i j)"),
                            masks_sb[:, mi:mi + 2, :].rearrange(
                                "p i j -> p (i j)"))
                _pv_dn(npair - 1)
                rc = p2.tile([1, 512], F32, tag="rc", bufs=2, name="rc")
                nc.vector.reciprocal(rc[:], dn[0:1, :])
                bc = p2.tile([128, 512], F32, tag="bc", bufs=2, name="bc")
                nc.gpsimd.partition_broadcast(bc[:], rc[:])
                at = p2.tile([128, 512], BF16, tag="at", bufs=3, name="at")
                nc.vector.tensor_mul(at[:], ov[:], bc[:])
                j = 4 * b + tqb
                nc.sync.dma_start(
                    out=cc_in[hl][j * 128:(j + 1) * 128, :], in_=at[:])
            if li == 1:  # both head0 lanes done -> first A2A
                nc.gpsimd.collective_compute(
                    "AllToAll", mybir.AluOpType.bypass,
                    replica_groups=[list(range(NCORES))],
                    ins=[cc_in[0][:]], outs=[cc_out[0][:]])
        nc.gpsimd.collective_compute(
            "AllToAll", mybir.AluOpType.bypass,
            replica_groups=[list(range(NCORES))],
            ins=[cc_in[1][:]], outs=[cc_out[1][:]])

    # ---- Phase 3: output projection for my 512 flattened rows ----
    # A_all channel blocks: pass l=0 -> global heads 0,2,..,14 (A2A#1),
    # pass l=1 -> heads 1,3,..,15 (A2A#2). wo_sb rows host-permuted to match.
    with tc.tile_pool(name="p3", bufs=1) as p3, \
         tc.tile_pool(name="psum3", bufs=1, space="PSUM") as psum3:
        a_sb = [None, None]
        part = [[None] * 2 for _ in range(4)]  # [tqt][cwh] f32 partials
        for l in range(HPC):
            a_sb[l] = p3.tile([128, NCORES, 512], BF16, name=f"a_sb{l}")
            cv = cc_out[l][:].rearrange("(ct p) t -> p ct t", p=128)
            for i in range(NCORES):
                nc.sync.dma_start(out=a_sb[l][:, i, :], in_=cv[:, i, :])
            for tqt in range(4):
                for cwh in range(2):
                    po = psum3.tile([128, 2, 512], F32, tag="po", bufs=3,
                                    name="po")
                    for ct in range(NCORES):
                        for cwg in range(2):
                            cw0 = cwh * 1024 + cwg * 512
                            nc.tensor.matmul(
                                po[:, cwg, :],
                                lhsT=a_sb[l][:, ct,
                                             tqt * 128:(tqt + 1) * 128],
                                rhs=wo_sb[:, l * NCORES + ct,
                                          cw0:cw0 + 512],
                                start=(ct == 0), stop=(ct == NCORES - 1))
                    if l == 0:
                        pp = p3.tile([128, 2, 512], F32,
                                     tag=f"part{tqt}_{cwh}", bufs=1,
                                     name=f"part{tqt}_{cwh}")
                        part[tqt][cwh] = pp
                        nc.vector.tensor_copy(pp[:], po[:])
                    else:
                        ot = p3.tile([128, 2, 512], F32, tag="ot", bufs=3,
                                     name="ot")
                        nc.vector.tensor_add(ot[:], po[:],
                                             part[tqt][cwh][:])
                        nc.sync.dma_start(
                            out=out_p[tqt * 128:(tqt + 1) * 128,
                                      cwh * 1024:(cwh + 1) * 1024],
                            in_=ot[:].rearrange("p i j -> p (i j)"))


def _build(reps: int = 1):
    nc = bacc.Bacc("TRN2", target_bir_lowering=False, debug=False,
                   num_devices=NCORES)

    params = {}
    params["xt"] = [nc.declare_dram_parameter(f"xt{b}", [D, T], BF16,
                                              isOutput=False)
                    for b in range(B)]
    for w in ("wq", "wk", "wv"):
        params[w] = nc.declare_dram_parameter(w, [D, CW], BF16,
                                              isOutput=False)
    params["wo"] = nc.declare_dram_parameter("wo", [D, D], BF16,
                                             isOutput=False)
    params["masks"] = nc.declare_dram_parameter("masks", [4, 128, 512], BF16,
                                                isOutput=False)
    params["out"] = nc.declare_dram_parameter("out", [RPC, D], F32,
                                              isOutput=True)

    with tile.TileContext(nc) as tc:
        with tc.tile_pool(name="consts", bufs=1) as consts, \
             tc.tile_pool(name="qkv", bufs=1) as qkv, \
             tc.tile_pool(name="dram", bufs=1, space="DRAM") as dram:

            for w, nm in (("wq", "wq_sb"), ("wk", "wk_sb"), ("wv", "wv_sb")):
                params[nm] = consts.tile([128, NDT, CW], BF16, name=nm)
            params["wo_sb"] = consts.tile([128, NDT, D], BF16, name="wo_sb")
            masks_sb = consts.tile([128, 4, 512], BF16, name="masks_sb")
            nc.sync.dma_start(out=masks_sb[:],
                              in_=params["masks"][:].rearrange(
                                  "i p j -> p i j"))
            params["masks_sb"] = masks_sb
            ones_col = consts.tile([128, 1], BF16, name="ones_col")
            nc.vector.memset(ones_col[:], 1.0)
            params["ones_col"] = ones_col

            # channel-major Q.T/K.T + natural V, resident through phase 2
            params["qh_sb"] = qkv.tile([128, B * HPC, T], BF16, name="qh_sb")
            params["kh_sb"] = qkv.tile([128, B * HPC, T], BF16, name="kh_sb")
            params["v_sb"] = qkv.tile([128, B * NTK, CW], BF16, name="v_sb")

            for rep in range(reps):
                _emit_rep(nc, tc, dram, params, rep)

    nc.compile()
    return nc


def _get_nc(reps: int = 1):
    key = f"nc{reps}"
    if key not in _CACHE:
        _CACHE[key] = _build(reps)
    return _CACHE[key]


def make_in_maps(query, Wq, Wk, Wv, Wo):
    """Per-core input maps (shared host-side prep for kernel() and test)."""
    query = np.asarray(query, dtype=np.float32)
    xt = [np.ascontiguousarray(query[b].T).astype(BF) for b in range(B)]
    p_idx = np.arange(128)[:, None]
    j_idx = np.arange(512)[None, :]
    masks = np.stack([(p_idx <= j_idx - 128 * i) for i in range(4)]
                     ).astype(BF)
    # Wo rows (input channels) permuted to the A2A channel-block order:
    # heads [0,2,...,14, 1,3,...,15]
    woT = np.ascontiguousarray(np.asarray(Wo, np.float32).T)
    perm = [h for h in range(0, H, 2)] + [h for h in range(1, H, 2)]
    wo_perm = np.concatenate([woT[128 * h:128 * (h + 1)] for h in perm],
                             axis=0).astype(BF)
    in_maps = []
    for c in range(NCORES):
        sl = slice(CW * c, CW * (c + 1))
        in_maps.append({
            "xt0": xt[0],
            "xt1": xt[1],
            "wq": np.ascontiguousarray(np.asarray(Wq, np.float32)[sl].T
                                       ).astype(BF),
            "wk": np.ascontiguousarray(np.asarray(Wk, np.float32)[sl].T
                                       ).astype(BF),
            "wv": np.ascontiguousarray(np.asarray(Wv, np.float32)[sl].T
                                       ).astype(BF),
            "wo": wo_perm,
            "masks": masks,
        })
    return in_maps


def kernel(query, attention_mask, Wq, Wk, Wv, Wo, bo):
    nc = _get_nc()
    in_maps = make_in_maps(query, Wq, Wk, Wv, Wo)
    res = run_bass_kernel_spmd(nc, in_maps, list(range(NCORES))).results
    flat = np.concatenate([res[c]["out"] for c in range(NCORES)], axis=0)
    out = flat.reshape(B, T, D) + np.asarray(bo, np.float32)[None, None, :]
    return out.astype(np.float32)
